# revision 26
# baseline (speedup 1.0000x reference)
"""Causal self-attention (B=2, T=2048, C=1024, H=16, D=64) on 8 trn2 cores.

Sharding: core c handles batch b = c//4 and head group hg = c%4 (heads
4*hg .. 4*hg+3).  Each core computes q/k/v projections for its 4 heads
(as 2 stacked head pairs), causal-softmax attention, and a partial
output projection y_partial = O_heads @ Wo[:, heads].T.  The host sums
the 4 partials per batch and adds the bias.

On-core layout (fp32r matmul operands, fp32 accumulation):
  qT/kT per pair: 4 chunk tiles [128, 512], rows 0:64 head-even,
          64:128 head-odd
  S^T_j = kT[j].T @ qT[I]   (k-major, K=64, both heads of a pair packed
          on PE row tiles 0/64)
  P = exp(S * 1/sqrt(C)) (ACT, batched over pairs of j) * causal mask
  O^T_aug = sum_j Vaug_j.T @ P_j    Vaug = [V_h | ones] -> row 64 of the
          [65, 512] PSUM accumulator is the softmax denominator
  O^T norm: reciprocal (DVE) -> partition_broadcast (GPSIMD) -> mul (DVE)
  y = sum_pairs (O^T stacked).T @ woT  (K=256 over 2 K-tiles of 128)

Emission is interleaved per 512-column chunk (projection chunk c, then
attention for I=c) because the Tile scheduler orders each engine's
instructions by emission priority; y-projection chunks are queued as PE
filler between attention steps (the attention inner loop is ACT-bound).
"""
import numpy as np

import concourse.tile as tile
import concourse.mybir as mybir
from concourse import bacc
from concourse.bass_utils import run_bass_kernel_spmd

FP = mybir.dt.float32
FPR = mybir.dt.float32r
B, T, C = 2, 2048, 1024
H, D = 16, 64
SCALE = 1.0 / 32.0  # 1/sqrt(C)
N_CORES = 8
NKT = C // 128  # 8 K-tiles over the embedding dim
NTK = T // 128  # 16 Tk tiles
NI = T // 512  # 4 Tq chunks
EXP = mybir.ActivationFunctionType.Exp

_nc_cache = {}


def _r(ap):
    """fp32r view of an fp32-layout AP (constants)."""
    return ap.bitcast(FPR)


def build_kernel(repeats=1, hmix=False):
    key = (repeats, hmix)
    if key in _nc_cache:
        return _nc_cache[key]

    nc = bacc.Bacc("TRN2", target_bir_lowering=False, debug=False)

    xT_d = nc.dram_tensor("xT", [C, T], FPR, kind="ExternalInput").ap()
    wqT_d = nc.dram_tensor("wqT", [C, 256], FPR, kind="ExternalInput").ap()
    wkT_d = nc.dram_tensor("wkT", [C, 256], FPR, kind="ExternalInput").ap()
    wvT_d = nc.dram_tensor("wvT", [C, 256], FPR, kind="ExternalInput").ap()
    woT_d = nc.dram_tensor("woT", [256, C], FPR, kind="ExternalInput").ap()
    y_d = nc.dram_tensor("y", [T, C], FP, kind="ExternalOutput").ap()

    # mask_big[p, y] = 1 iff y >= p + 384 : slice [., c0:c0+z+128] with
    # c0 = 384 - z, z = j*128 - I*512 masks diagonal tile j (cols < z are
    # fully below the causal boundary, the next 128 are triangular)
    mask_np = (
        np.arange(896)[None, :] >= (np.arange(128)[:, None] + 384)
    ).astype(np.float32)
    mask_d = nc.inline_tensor(mask_np, "mask_big").ap()
    ident_d = nc.inline_tensor(np.eye(128, dtype=np.float32), "ident").ap()
    ones_d = nc.inline_tensor(np.ones((128, 1), dtype=np.float32), "ones").ap()

    with tile.TileContext(nc) as tc:
        with (
            tc.tile_pool(name="persist", bufs=1) as pp,
            tc.tile_pool(name="xpool", bufs=16) as xpool,
            tc.tile_pool(name="ppool", bufs=4) as ppool,
            tc.tile_pool(name="spool", bufs=4) as spool,
            tc.tile_pool(name="ypool", bufs=4) as ypool,
            tc.tile_pool(name="ps_s", bufs=2, space="PSUM") as ps_s,
            tc.tile_pool(name="ps_o", bufs=2, space="PSUM") as ps_o,
            tc.tile_pool(name="ps_y", bufs=2, space="PSUM") as ps_y,
        ):
            # ---- critical-path DMAs first: wq, then xT chunk 0, then the
            # rest; weight matrices load as ONE rearranged DMA each to keep
            # the serial HWDGE issue path short ----
            wq_big = pp.tile([128, NKT, 256], FPR, tag="wq")
            nc.sync.dma_start(
                wq_big[:, :, :], wqT_d.rearrange("(n p) d -> p n d", p=128)
            )
            wq = [wq_big[:, kk, :] for kk in range(NKT)]
            xts_by_chunk = {0: [None] * NKT, 1: [None] * NKT}
            for kk in range(NKT):
                xt = xpool.tile([128, 512], FPR, tag="xt", name=f"xtc0_{kk}")
                nc.sync.dma_start(xt[:], xT_d[kk * 128 : (kk + 1) * 128, 0:512])
                xts_by_chunk[0][kk] = xt
            wk_big = pp.tile([128, NKT, 256], FPR, tag="wk")
            nc.sync.dma_start(
                wk_big[:, :, :], wkT_d.rearrange("(n p) d -> p n d", p=128)
            )
            wk = [wk_big[:, kk, :] for kk in range(NKT)]
            wv_big = pp.tile([128, NKT, 256], FPR, tag="wv")
            nc.sync.dma_start(
                wv_big[:, :, :], wvT_d.rearrange("(n p) d -> p n d", p=128)
            )
            wv = [wv_big[:, kk, :] for kk in range(NKT)]
            for kk in range(NKT):
                xt = xpool.tile([128, 512], FPR, tag="xt", name=f"xtc1_{kk}")
                nc.sync.dma_start(xt[:], xT_d[kk * 128 : (kk + 1) * 128, 512:1024])
                xts_by_chunk[1][kk] = xt
            wo_big = pp.tile([128, 2, C], FPR, tag="wo")
            nc.sync.dma_start(
                wo_big[:, :, :], woT_d.rearrange("(n p) d -> p n d", p=128)
            )
            wo = [wo_big[:, kk, :] for kk in range(2)]

            mask = pp.tile([128, 896], FP, tag="mask")
            nc.sync.dma_start(mask[:], mask_d[:])
            ones_sb = pp.tile([128, 1], FPR, tag="ones")
            nc.sync.dma_start(ones_sb[:], _r(ones_d[:]))
            ident = pp.tile([128, 128], FP, tag="ident")
            nc.sync.dma_start(ident[:], ident_d[:])

            # ---- persistent activations, chunked per 512 columns ----
            qTc = [
                [pp.tile([128, 512], FPR, tag=f"qT{p}_{i}", name=f"qT{p}_{i}")
                 for i in range(NI)]
                for p in range(2)
            ]
            kTc = [
                [pp.tile([128, 512], FPR, tag=f"kT{p}_{i}", name=f"kT{p}_{i}")
                 for i in range(NI)]
                for p in range(2)
            ]
            vTc = [
                [pp.tile([128, 512], FP, tag=f"vT{p}_{i}", name=f"vT{p}_{i}")
                 for i in range(NI)]
                for p in range(2)
            ]
            otstc = [
                [pp.tile([128, 512], FPR, tag=f"ot{p}_{i}", name=f"otst{p}_{i}")
                 for i in range(NI)]
                for p in range(2)
            ]
            vaug = [
                [
                    pp.tile([128, 130], FPR, tag=f"va{p}_{t}", name=f"vaug{p}_{t}")
                    for t in range(NTK)
                ]
                for p in range(2)
            ]

            # ---- emission helpers ----
            R = [0]
            def emit_xt_chunk(c):
                tiles = []
                for kk in range(NKT):
                    xt = xpool.tile([128, 512], FPR, tag="xt", name=f"xtc{c}_{kk}_r{R[0]}")
                    nc.sync.dma_start(
                        xt[:], xT_d[kk * 128 : (kk + 1) * 128, c * 512 : c * 512 + 512]
                    )
                    tiles.append(xt)
                return tiles

            def emit_proj_chunk(c, xts):
                for wts, dsts, nm in ((wq, qTc, "q"), (wk, kTc, "k"), (wv, vTc, "v")):
                    for pair in range(2):
                        ps = ps_y.tile([128, 512], FP, tag="ps_proj",
                                       name=f"pspr{nm}{c}_{pair}_r{R[0]}")
                        for kk in range(NKT):
                            nc.tensor.matmul(
                                ps[:],
                                lhsT=wts[kk][:, pair * 128 : pair * 128 + 128],
                                rhs=xts[kk][:],
                                start=(kk == 0),
                                stop=(kk == NKT - 1),
                            )
                        nc.vector.tensor_copy(dsts[pair][c][:], ps[:])

            def emit_transposes(c):
                for pair in range(2):
                    for t in range(4 * c, 4 * c + 4):
                        pst = ps_y.tile([128, 128], FP, tag="ps_proj",
                                        name=f"pstr{pair}_{t}_r{R[0]}")
                        nc.tensor.transpose(
                            pst[:],
                            vTc[pair][c][:, (t % 4) * 128 : (t % 4) * 128 + 128],
                            ident[:],
                        )
                        va = vaug[pair][t]
                        nc.vector.tensor_copy(va[:, 64:65], ones_sb[:])
                        nc.vector.tensor_copy(va[:, 129:130], ones_sb[:])
                        if c < 2:  # ACT is idle before attention starts
                            nc.scalar.copy(va[:, 0:64], pst[:, 0:64])
                            nc.scalar.copy(va[:, 65:129], pst[:, 64:128])
                        else:  # mid-attention: ACT is the exp bottleneck
                            nc.vector.tensor_copy(va[:, 0:64], pst[:, 0:64])
                            nc.vector.tensor_copy(va[:, 65:129], pst[:, 64:128])

            fillers = []

            def emit_yproj_chunk(t, nch, on_act=False):
                ps = ps_y.tile([128, 512], FP, tag="ps_proj", name=f"psy{t}_{nch}_r{R[0]}")
                for pair in range(2):
                    nc.tensor.matmul(
                        ps[:],
                        lhsT=otstc[pair][t // 4][
                            :, (t % 4) * 128 : (t % 4) * 128 + 128
                        ],
                        rhs=wo[pair][:, nch * 512 : nch * 512 + 512],
                        start=(pair == 0),
                        stop=(pair == 1),
                    )
                yt = ypool.tile([128, 512], FP, tag="yout", name=f"yt{t}_{nch}_r{R[0]}")
                if on_act:
                    nc.scalar.copy(yt[:], ps[:])
                else:
                    nc.vector.tensor_copy(yt[:], ps[:])
                nc.sync.dma_start(
                    y_d[t * 128 : (t + 1) * 128, nch * 512 : nch * 512 + 512],
                    yt[:],
                )

            nfill = [0]

            def maybe_fill():
                nfill[0] += 1
                if nfill[0] % 2 == 0 and fillers:
                    fillers.pop(0)()

            def emit_attention(I):
                if hmix:
                    emit_attention_hmix(I)
                    return
                jmax = 4 * I + 4
                for pair in range(2):
                    oT = [None, None]
                    for h in (1, 0):
                        oT[h] = ps_o.tile([65, 512], FP, tag="oT",
                                          name=f"o{I}_{pair}_{h}_r{R[0]}")
                        hsl = slice(64 * h, 64 * h + 64)
                        for jb in range(jmax // 2):
                            j0 = 2 * jb
                            diag = j0 >= 4 * I  # both tiles in diagonal region
                            zs = [max(0, (j0 + dj) * 128 - I * 512) for dj in range(2)]
                            s_ps = ps_s.tile([128, 1024], FP, tag="s",
                                             name=f"s{I}_{pair}_{h}_{jb}_r{R[0]}")
                            for dj in range(2):
                                j = j0 + dj
                                z = zs[dj]
                                nc.tensor.matmul(
                                    s_ps[:, dj * 512 + z : dj * 512 + 512],
                                    lhsT=kTc[pair][j // 4][
                                        hsl, (j % 4) * 128 : (j % 4) * 128 + 128
                                    ],
                                    rhs=qTc[pair][I][hsl, z:512],
                                    start=True,
                                    stop=True,
                                )
                            p_sb = ppool.tile([128, 1024], FPR, tag="p",
                                              name=f"p{I}_{pair}_{h}_{jb}_r{R[0]}")
                            if not diag:
                                nc.scalar.activation(p_sb[:], s_ps[:], EXP,
                                                     scale=SCALE)
                            else:
                                # trimmed: columns below the causal boundary
                                # were never computed
                                for dj in range(2):
                                    lo = dj * 512 + zs[dj]
                                    hi = dj * 512 + 512
                                    nc.scalar.activation(
                                        p_sb[:, lo:hi], s_ps[:, lo:hi], EXP,
                                        scale=SCALE,
                                    )
                            for dj in range(2):
                                j = j0 + dj
                                z = zs[dj]
                                if j >= 4 * I:
                                    # triangular strip at the causal boundary
                                    ssl2 = slice(dj * 512 + z, dj * 512 + z + 128)
                                    nc.vector.tensor_mul(
                                        p_sb[:, ssl2], p_sb[:, ssl2],
                                        _r(mask[:, 384:512]),
                                    )
                                nc.tensor.matmul(
                                    oT[h][:, z:512],
                                    lhsT=vaug[pair][j][:, 65 * h : 65 * h + 65],
                                    rhs=p_sb[:, dj * 512 + z : dj * 512 + 512],
                                    start=(j == 0),
                                    stop=(j == jmax - 1),
                                )
                            maybe_fill()
                    # normalize: O^T[0:64] * (1/rowsum) into the stacked chunk
                    for h in (1, 0):
                        recip = spool.tile([1, 512], FP, tag="recip",
                                           name=f"rc{I}_{pair}_{h}_r{R[0]}")
                        nc.vector.reciprocal(recip[:], oT[h][64:65, :])
                        bcast = spool.tile([64, 512], FP, tag="bcast",
                                           name=f"bc{I}_{pair}_{h}_r{R[0]}")
                        nc.gpsimd.partition_broadcast(bcast[:], recip[:])
                        if h == 0:
                            nc.vector.tensor_mul(
                                otstc[pair][I][0:64, :], oT[h][0:64, :], bcast[:]
                            )
                        else:
                            onrm = spool.tile([64, 512], FPR, tag="onrm",
                                              name=f"on{I}_{pair}_r{R[0]}")
                            nc.vector.tensor_mul(onrm[:], oT[h][0:64, :], bcast[:])
                            # partition shift 0->64 needs a DMA
                            nc.sync.dma_start(otstc[pair][I][64:128, :], onrm[:])
                for t in range(4 * I, 4 * I + 4):
                    for nch in range(2):
                        fillers.append(
                            lambda t=t, nch=nch, **kw: emit_yproj_chunk(t, nch, **kw)
                        )

            def emit_attention_hmix(I):
                # Both heads of a pair advance together so that the two K=64
                # S matmuls (PE row groups 0 and 64) are adjacent in the PE
                # stream and can overlap on hardware.
                jmax = 4 * I + 4
                for pair in range(2):
                    oT = []
                    for h in range(2):
                        o = ps_o.tile([65, 512], FP, tag="oT",
                                      name=f"o{I}_{pair}_{h}_r{R[0]}")
                        oT.append(o)
                    for jb in range(jmax // 2):
                        j0 = 2 * jb
                        s_ps = []
                        for h in range(2):
                            sp = ps_s.tile([128, 1024], FP, tag="s",
                                           name=f"s{I}_{pair}_{h}_{jb}_r{R[0]}")
                            s_ps.append(sp)
                        for dj in range(2):
                            j = j0 + dj
                            for h in range(2):
                                hsl = slice(64 * h, 64 * h + 64)
                                nc.tensor.matmul(
                                    s_ps[h][:, dj * 512 : dj * 512 + 512],
                                    lhsT=kTc[pair][j // 4][
                                        hsl, (j % 4) * 128 : (j % 4) * 128 + 128
                                    ],
                                    rhs=qTc[pair][I][hsl, :],
                                    start=True,
                                    stop=True,
                                )
                        p_sb = []
                        for h in range(2):
                            pt = ppool.tile([128, 1024], FPR, tag="p",
                                            name=f"p{I}_{pair}_{h}_{jb}_r{R[0]}")
                            nc.scalar.activation(pt[:], s_ps[h][:], EXP, scale=SCALE)
                            p_sb.append(pt)
                        for dj in range(2):
                            j = j0 + dj
                            for h in range(2):
                                if j >= 4 * I:  # diagonal tile: causal mask
                                    z = j * 128 - I * 512
                                    c0 = 384 - z
                                    msl = slice(dj * 512, dj * 512 + z + 128)
                                    nc.vector.tensor_mul(
                                        p_sb[h][:, msl], p_sb[h][:, msl],
                                        _r(mask[:, c0 : c0 + z + 128]),
                                    )
                                nc.tensor.matmul(
                                    oT[h][:],
                                    lhsT=vaug[pair][j][:, 65 * h : 65 * h + 65],
                                    rhs=p_sb[h][:, dj * 512 : dj * 512 + 512],
                                    start=(j == 0),
                                    stop=(j == jmax - 1),
                                )
                        maybe_fill()
                        maybe_fill()
                    for h in (1, 0):
                        recip = spool.tile([1, 512], FP, tag="recip",
                                           name=f"rc{I}_{pair}_{h}_r{R[0]}")
                        nc.vector.reciprocal(recip[:], oT[h][64:65, :])
                        bcast = spool.tile([64, 512], FP, tag="bcast",
                                           name=f"bc{I}_{pair}_{h}_r{R[0]}")
                        nc.gpsimd.partition_broadcast(bcast[:], recip[:])
                        if h == 0:
                            nc.vector.tensor_mul(
                                otstc[pair][I][0:64, :], oT[h][0:64, :], bcast[:]
                            )
                        else:
                            onrm = spool.tile([64, 512], FPR, tag="onrm",
                                              name=f"on{I}_{pair}_r{R[0]}")
                            nc.vector.tensor_mul(onrm[:], oT[h][0:64, :], bcast[:])
                            nc.sync.dma_start(otstc[pair][I][64:128, :], onrm[:])
                for t in range(4 * I, 4 * I + 4):
                    for nch in range(2):
                        fillers.append(
                            lambda t=t, nch=nch, **kw: emit_yproj_chunk(t, nch, **kw)
                        )

            # ---- interleaved emission: proj chunk c, then attention I=c;
            # attention I=0 (shortest) is slotted after I=2 ----
            for rep in range(repeats):
                R[0] = rep
                for c in range(NI):
                    if rep == 0 and c in xts_by_chunk:
                        xts = xts_by_chunk[c]
                    else:
                        xts = emit_xt_chunk(c)
                    emit_proj_chunk(c, xts)
                    emit_transposes(c)
                    if c >= 1:
                        emit_attention(c)
                    if c == 2:
                        emit_attention(0)
                while fillers:
                    fillers.pop(0)(on_act=True)  # tail: ACT is idle here

    nc.compile()
    _nc_cache[key] = nc
    return nc


def make_in_maps(x, Wq, Wk, Wv, Wo):
    x = np.asarray(x, dtype=np.float32)
    Wq = np.asarray(Wq, dtype=np.float32)
    Wk = np.asarray(Wk, dtype=np.float32)
    Wv = np.asarray(Wv, dtype=np.float32)
    Wo = np.asarray(Wo, dtype=np.float32)
    in_maps = []
    for c in range(N_CORES):
        b, hg = c // 4, c % 4
        sl = slice(256 * hg, 256 * hg + 256)
        in_maps.append(
            {
                "xT": np.ascontiguousarray(x[b].T),
                "wqT": np.ascontiguousarray(Wq[sl, :].T),
                "wkT": np.ascontiguousarray(Wk[sl, :].T),
                "wvT": np.ascontiguousarray(Wv[sl, :].T),
                "woT": np.ascontiguousarray(Wo[:, sl].T),
            }
        )
    return in_maps


def run_spmd(in_maps, trace=False, repeats=1, **kw):
    nc = build_kernel(repeats)
    return run_bass_kernel_spmd(nc, in_maps, list(range(N_CORES)), trace=trace, **kw)


def gather(results, bo):
    bo = np.asarray(bo, dtype=np.float32)
    y = np.empty((B, T, C), dtype=np.float32)
    for b in range(B):
        acc = results[4 * b]["y"].astype(np.float32).copy()
        for g in range(1, 4):
            acc += results[4 * b + g]["y"]
        y[b] = acc + bo[None, :]
    return y


def kernel(x, Wq, Wk, Wv, Wo, bo):
    res = run_spmd(make_in_maps(x, Wq, Wk, Wv, Wo))
    return gather(res.results, bo)
